# revision 25
# baseline (speedup 1.0000x reference)
"""Multi-head attention (B=4, S=1024, D=1024, H=16, E=64) for 8 TRN2 NeuronCores.

Sharding: core c handles batch b = c//2 and heads 8*(c%2) .. 8*(c%2)+8.
Each core computes a partial output projection over its 8 heads; the host sums
the two partials per batch and adds the bias.

Per-core kernel (Bass/Tile):
  - QKV: qT/kT produced transposed [(2 heads x 64e)=128, S]; v natural [t, he].
    Scale sqrt(D)=32 folded into Wq on the host.
  - scores[s,t] = qT.T @ kT per head, two heads packed in the PE array via
    row-tiling (K=64 each at partition offsets 0/64).
  - causal mask added on the diagonal 128-block, row-max via DVE reduce
    (negate), exp via ACT activation (bias=-max, accum_out=denominator).
  - E (unnormalized probs) transposed [s,t]->[t,s] via bf16 DMA transpose
    (fast mode) or fp32 PE transpose (safe mode).
  - out.T[e,s] = sum_t v[t,e] E_T[t,s], two heads packed via PE col-tiling.
  - normalization (1/denominator, per head per s) folded in after AV:
    out.T scaled by a DMA-broadcast reciprocal row.
  - projection y[s,:] = sum_pairs oT_pair.T @ WpT_pair.
"""
import os
import numpy as np
from contextlib import ExitStack

import concourse.bass as bass
import concourse.bacc as bacc
import concourse.tile as tile
import concourse.mybir as mybir
from concourse.bass_utils import run_bass_kernel_spmd

F32 = mybir.dt.float32
F32R = mybir.dt.float32r
BF16 = mybir.dt.bfloat16
AX = mybir.AxisListType
ALU = mybir.AluOpType
ACTF = mybir.ActivationFunctionType

B, S, D = 4, 1024, 1024
H, E = 16, 64
P = 128
NK = D // P          # 8 contraction chunks
NS = S // P          # 8 s-tiles
HL = 8               # heads per core
NPAIR = HL // 2      # 4 head pairs
NEG_BIG = -1.0e30

DEBUG = False

# config knobs
QK_F32R = True       # f32r for QKV projections + scores matmuls
AV_BF16 = True       # bf16 E + v (else f32r E + v)
TRANSPOSE_DMA = False  # DMA-transpose E (bf16 only) vs PE transpose
PROJ_F32R = True     # f32r for output projection


def emit_kernel(tc):
    nc = tc.nc
    ctx = ExitStack()

    QK = F32R if QK_F32R else F32
    PJ = F32R if PROJ_F32R else F32

    xT = nc.dram_tensor("xT", [D, S], QK, kind="ExternalInput").ap()
    wq_d = nc.dram_tensor("wq", [NK, P, 512], QK, kind="ExternalInput").ap()
    wk_d = nc.dram_tensor("wk", [NK, P, 512], QK, kind="ExternalInput").ap()
    wv_d = nc.dram_tensor("wv", [NK, P, 512], QK, kind="ExternalInput").ap()
    wpT_d = nc.dram_tensor("wpT", [NPAIR, P, D], PJ, kind="ExternalInput").ap()
    mask_d = nc.dram_tensor("mask", [P, P], F32, kind="ExternalInput").ap()
    patt_d = nc.dram_tensor("patt", [2, P], F32, kind="ExternalInput").ap()
    ident_d = nc.dram_tensor("ident", [P, P], F32, kind="ExternalInput").ap()
    identr_d = nc.dram_tensor(
        "identr", [P, P], BF16 if AV_BF16 else QK, kind="ExternalInput").ap()
    y_d = nc.dram_tensor("y", [S, D], F32, kind="ExternalOutput").ap()

    dbg = {}
    if DEBUG:
        def dump(name, ap):
            d = nc.dram_tensor(f"dbg_{name}", ap.shape, ap.dtype,
                               kind="ExternalOutput").ap()
            nc.sync.dma_start(d[:], ap[:])
    else:
        def dump(name, ap):
            pass

    # ---------------- pools ----------------
    const_p = ctx.enter_context(tc.tile_pool(name="const", bufs=1))
    qk_p = ctx.enter_context(tc.tile_pool(name="qk", bufs=1))
    o_p = ctx.enter_context(tc.tile_pool(name="o", bufs=1))
    y_p = ctx.enter_context(tc.tile_pool(name="y", bufs=3))
    st_p = ctx.enter_context(tc.tile_pool(name="st", bufs=8))
    rep_p = ctx.enter_context(tc.tile_pool(name="rep", bufs=4))
    w_p = tc.alloc_tile_pool(name="w", bufs=1)

    ps_mm = ctx.enter_context(tc.tile_pool(name="psmm", bufs=2, space="PSUM"))
    ps_sc = ctx.enter_context(tc.tile_pool(name="pssc", bufs=3, space="PSUM"))
    ps_sm = ctx.enter_context(tc.tile_pool(name="pssm", bufs=1, space="PSUM"))

    # ---------------- load inputs ----------------
    mask_sb = const_p.tile([P, P], F32)
    nc.sync.dma_start(mask_sb[:], mask_d[:])
    ident_sb = const_p.tile([P, P], F32)
    nc.sync.dma_start(ident_sb[:], ident_d[:])
    patt_sb = const_p.tile([2, P], F32)
    nc.sync.dma_start(patt_sb[:], patt_d[:])
    ident_e = const_p.tile([P, P], BF16 if AV_BF16 else QK, name="ident_e")
    nc.sync.dma_start(ident_e[:], identr_d[:])

    xt = w_p.tile([P, NK, S], QK)
    for k in range(NK):
        nc.sync.dma_start(xt[:, k, :], xT[k * P:(k + 1) * P, :])

    wq_sb = w_p.tile([P, NK, 512], QK)
    wk_sb = w_p.tile([P, NK, 512], QK)
    wv_sb = w_p.tile([P, NK, 512], QK)
    for k in range(NK):
        nc.sync.dma_start(wq_sb[:, k, :], wq_d[k])
        nc.sync.dma_start(wk_sb[:, k, :], wk_d[k])
        nc.sync.dma_start(wv_sb[:, k, :], wv_d[k])
    wpT_sb = const_p.tile([P, NPAIR, D], PJ)
    for pr in range(NPAIR):
        nc.sync.dma_start(wpT_sb[:, pr, :], wpT_d[pr])

    # ---------------- QKV ----------------
    # qT/kT: [128=(head01, e), S] per pair; v natural [t, (pair, head01, e)]
    qT = [qk_p.tile([P, S], QK, name=f"qT{pr}") for pr in range(NPAIR)]
    kT = [qk_p.tile([P, S], QK, name=f"kT{pr}") for pr in range(NPAIR)]
    v_dt = BF16 if AV_BF16 else QK
    v_sb = qk_p.tile([P, NS, 512], v_dt)

    for pr in range(NPAIR):
        for dst, w_t in ((qT, wq_sb), (kT, wk_sb)):
            for sg in range(2):
                ps = ps_mm.tile([P, 512], F32, tag="mm")
                for k in range(NK):
                    nc.tensor.matmul(
                        ps[:],
                        w_t[:, k, pr * P:(pr + 1) * P],
                        xt[:, k, sg * 512:(sg + 1) * 512],
                        start=(k == 0), stop=(k == NK - 1),
                    )
                nc.scalar.copy(dst[pr][:, sg * 512:(sg + 1) * 512], ps[:])
    for st in range(NS):
        ps = ps_mm.tile([P, 512], F32, tag="mm")
        for k in range(NK):
            nc.tensor.matmul(
                ps[:],
                xt[:, k, st * P:(st + 1) * P],
                wv_sb[:, k, :],
                start=(k == 0), stop=(k == NK - 1),
            )
        nc.scalar.copy(v_sb[:, st, :], ps[:])

    dump("qT0", qT[0])
    dump("kT0", kT[0])
    dump("qT1", qT[1])
    dump("kT1", kT[1])
    dump("v", v_sb)

    # ---------------- attention ----------------
    w_p.release()
    e_p = ctx.enter_context(tc.tile_pool(name="e", bufs=4))
    et_p = ctx.enter_context(tc.tile_pool(name="et", bufs=2))
    e_dt = BF16 if AV_BF16 else QK
    den = [st_p.tile([P, NS], F32, name=f"den{h}", tag="den") for h in range(HL)]
    et = {}

    def scores_softmax(pr):
        """scores + softmax + transpose for both heads of pair pr."""
        for h01 in range(2):
            h = 2 * pr + h01
            et[h] = et_p.tile([P, NK, S], e_dt, name=f"et{h}", tag="et")
            # zero the never-written regions (t-chunk j covers s >= 128j only)
            for j in range(1, NK):
                nc.gpsimd.memset(et[h][:, j, 0:j * P], 0.0)
        for i in range(NS):
            tlen = (i + 1) * P
            nch = (tlen + 511) // 512
            for h01 in range(2):
                h = 2 * pr + h01
                rows = slice(64 * h01, 64 * h01 + 64)
                e_sb = e_p.tile([P, S], e_dt, name=f"e{h}_{i}", tag="e")
                pss = []
                for c in range(nch):
                    clen = min(512, tlen - 512 * c)
                    ps = ps_sc.tile([P, clen], F32, tag="sc")
                    nc.tensor.matmul(
                        ps[:],
                        qT[pr][rows, i * P:(i + 1) * P],
                        kT[pr][rows, 512 * c:512 * c + clen],
                        start=True, stop=True,
                    )
                    pss.append((ps, clen))
                # causal mask on the diagonal 128 block (last 128 cols)
                ps_l, clen_l = pss[-1]
                nc.vector.tensor_add(
                    ps_l[:, clen_l - P:clen_l], ps_l[:, clen_l - P:clen_l], mask_sb[:])
                # row max (negated)
                negmax = st_p.tile([P, 1], F32, name=f"ngm{h}_{i}", tag="ngm")
                if nch == 1:
                    nc.vector.tensor_reduce(
                        negmax[:], pss[0][0][:], axis=AX.X, op=ALU.max, negate=True)
                else:
                    nm0 = st_p.tile([P, 1], F32, name=f"nm0{h}_{i}", tag="nm0")
                    nm1 = st_p.tile([P, 1], F32, name=f"nm1{h}_{i}", tag="nm1")
                    nc.vector.tensor_reduce(
                        nm0[:], pss[0][0][:], axis=AX.X, op=ALU.max, negate=True)
                    nc.vector.tensor_reduce(
                        nm1[:], pss[1][0][:], axis=AX.X, op=ALU.max, negate=True)
                    nc.vector.tensor_tensor(negmax[:], nm0[:], nm1[:], op=ALU.min)
                # exp(x - max), denominator via accum
                if nch == 1:
                    nc.scalar.activation(
                        e_sb[:, 0:tlen], pss[0][0][:], ACTF.Exp,
                        bias=negmax[:], scale=1.0,
                        accum_out=den[h][:, i:i + 1])
                else:
                    d0 = st_p.tile([P, 1], F32, name=f"d0{h}_{i}", tag="d0")
                    d1 = st_p.tile([P, 1], F32, name=f"d1{h}_{i}", tag="d1")
                    nc.scalar.activation(
                        e_sb[:, 0:512], pss[0][0][:], ACTF.Exp,
                        bias=negmax[:], scale=1.0, accum_out=d0[:])
                    nc.scalar.activation(
                        e_sb[:, 512:tlen], pss[1][0][:], ACTF.Exp,
                        bias=negmax[:], scale=1.0, accum_out=d1[:])
                    nc.vector.tensor_add(den[h][:, i:i + 1], d0[:], d1[:])
                if h == 0 and i == 1:
                    dump("e01", e_sb[:, 0:tlen])
                if h == 2 and i == 5:
                    dump("e25", e_sb[:, 0:tlen])
                if h == 2 and i == 6:
                    dump("e26", e_sb[:, 0:tlen])
                if h == 2 and i == 7:
                    dump("e27", e_sb[:, 0:tlen])
                # transpose E[s, 0:tlen] -> et[h][:, 0:i+1, s-block i]
                if TRANSPOSE_DMA:
                    # one DMA per 128-block: contiguous destination runs
                    for j in range(i + 1):
                        nc.sync.dma_start_transpose(
                            et[h][:, j, i * P:(i + 1) * P],
                            e_sb[:, j * P:(j + 1) * P])
                else:
                    for j in range(i + 1):
                        pst = ps_sm.tile([P, P], e_dt, tag="ett", bufs=2)
                        nc.tensor.transpose(
                            pst[:], e_sb[:, j * P:(j + 1) * P], ident_e[:])
                        nc.scalar.copy(et[h][:, j, i * P:(i + 1) * P], pst[:])
        if pr == 0:
            dump("et0", et[0])
            dump("et1", et[1])
        if pr == 1:
            dump("et2", et[2])
            dump("et3", et[3])

    recipT = [st_p.tile([2, S], F32, tag=f"recipT{pr}", bufs=1,
                        name=f"recipT{pr}") for pr in range(NPAIR)]
    recip_tmp = st_p.tile([NS, HL * P], F32, tag="rectmp", bufs=1)
    oT = [o_p.tile([P, S], PJ, name=f"oT{pr}") for pr in range(NPAIR)]

    def av(pr):
        for h01 in range(2):
            h = 2 * pr + h01
            # reciprocal of denominators, laid out as a row [1, S]
            psd = ps_sm.tile([NS, P], F32, tag="sm", bufs=1)
            nc.tensor.transpose(psd[:], den[h][:], ident_sb[:])
            nc.vector.reciprocal(recip_tmp[:, h * P:(h + 1) * P], psd[:])
            nc.sync.dma_start(recipT[pr][h01:h01 + 1, :],
                              recip_tmp[:, h * P:(h + 1) * P])
        for sg in range(2):
            jmax = 4 * (sg + 1)
            # separate PSUM banks per head: one accumulation group per
            # 2KB zero-region (two groups in one bank is UB on HW)
            psavA = ps_mm.tile([P, 512], F32, tag="mm", name=f"psavA{pr}_{sg}")
            psavB = ps_mm.tile([P, 512], F32, tag="mm", name=f"psavB{pr}_{sg}")
            for j in range(jmax):
                for h01 in range(2):
                    h = 2 * pr + h01
                    cols = slice(pr * P + 64 * h01, pr * P + 64 * h01 + 64)
                    lhs = v_sb[:, j, cols]
                    rhs = et[h][:, j, sg * 512:(sg + 1) * 512]
                    out = psavA[0:64, :] if h01 == 0 else psavB[64:128, :]
                    nc.tensor.matmul(
                        out, lhs, rhs,
                        start=(j == 0), stop=(j == jmax - 1),
                        tile_position=(0, 64 * h01),
                    )
            # normalization: replicate reciprocal rows via a K=2 f32 matmul
            psrep = ps_sm.tile([P, 512], F32, tag="sm", bufs=1,
                               name=f"psrep{pr}_{sg}")
            nc.tensor.matmul(
                psrep[:], patt_sb[:],
                recipT[pr][:, sg * 512:(sg + 1) * 512],
                start=True, stop=True)
            rep = rep_p.tile([P, 512], F32, tag="rep", name=f"rep{pr}_{sg}")
            nc.scalar.copy(rep[:], psrep[:])
            nc.vector.tensor_mul(
                oT[pr][0:64, sg * 512:(sg + 1) * 512],
                psavA[0:64, :], rep[0:64, :])
            nc.vector.tensor_mul(
                oT[pr][64:128, sg * 512:(sg + 1) * 512],
                psavB[64:128, :], rep[64:128, :])

    # interleave: scores(p) ... scores(p+1) emitted before av(p)
    # (debug dumps of den/recipT/oT happen after av below)
    scores_softmax(0)
    for pr in range(1, NPAIR):
        scores_softmax(pr)
        av(pr - 1)
    av(NPAIR - 1)

    dump("den0", den[0])
    dump("den2", den[2])
    dump("den3", den[3])
    dump("recipT0", recipT[0])
    dump("recipT1", recipT[1])
    dump("oT0", oT[0])
    dump("oT1", oT[1])
    dump("oT2", oT[2])
    dump("oT3", oT[3])
    if DEBUG:
        for pr in range(NPAIR):
            ydbg = nc.dram_tensor(f"dbg_y{pr}", [S, D], F32,
                                  kind="ExternalOutput").ap()
            for st in range(NS):
                for dg in range(2):
                    pw = ps_mm.tile([P, 512], F32, tag="mm",
                                    name=f"pw{pr}_{st}_{dg}")
                    nc.tensor.matmul(
                        pw[:],
                        oT[pr][:, st * P:(st + 1) * P],
                        wpT_sb[:, pr, dg * 512:(dg + 1) * 512],
                        start=True, stop=True)
                    yw = y_p.tile([P, 512], F32, tag="y",
                                  name=f"yw{pr}_{st}_{dg}")
                    nc.scalar.copy(yw[:], pw[:])
                    nc.sync.dma_start(
                        ydbg[st * P:(st + 1) * P, dg * 512:(dg + 1) * 512],
                        yw[:])

    # ---------------- projection ----------------
    for st in range(NS):
        for dg in range(2):
            psy = ps_mm.tile([P, 512], F32, tag="mm", name=f"psy{st}_{dg}")
            for pr in range(NPAIR):
                nc.tensor.matmul(
                    psy[:],
                    oT[pr][:, st * P:(st + 1) * P],
                    wpT_sb[:, pr, dg * 512:(dg + 1) * 512],
                    start=(pr == 0), stop=(pr == NPAIR - 1),
                )
            y_sb = y_p.tile([P, 512], F32, tag="y")
            nc.scalar.copy(y_sb[:], psy[:])
            nc.sync.dma_start(y_d[st * P:(st + 1) * P, dg * 512:(dg + 1) * 512],
                              y_sb[:])

    ctx.close()


_PROGRAM = None


def build_program():
    global _PROGRAM
    if _PROGRAM is None:
        nc = bacc.Bacc("TRN2")
        with tile.TileContext(nc) as tc:
            emit_kernel(tc)
        nc.compile()
        _PROGRAM = nc
    return _PROGRAM


def make_in_maps(x, Wq, Wk, Wv, Wp):
    """Build the 8 per-core input maps (core c: batch c//2, head-half c%2)."""
    x = np.asarray(x, dtype=np.float32)
    Wq = np.asarray(Wq, dtype=np.float32)
    Wk = np.asarray(Wk, dtype=np.float32)
    Wv = np.asarray(Wv, dtype=np.float32)
    Wp = np.asarray(Wp, dtype=np.float32)

    scale = np.float32(D) ** 0.5
    # [H, E, D] -> [D, H, E]
    wq_t = np.ascontiguousarray((Wq * scale).transpose(2, 0, 1))
    wk_t = np.ascontiguousarray(Wk.transpose(2, 0, 1))
    wv_t = np.ascontiguousarray(Wv.transpose(2, 0, 1))

    mask = np.where(np.tril(np.ones((P, P), dtype=bool)), np.float32(0.0),
                    np.float32(NEG_BIG)).astype(np.float32)
    ident = np.eye(P, dtype=np.float32)
    patt = np.zeros((2, P), dtype=np.float32)
    patt[0, 0:64] = 1.0
    patt[1, 64:128] = 1.0

    halves = []
    for half in range(2):
        hs = slice(8 * half, 8 * half + 8)
        halves.append({
            "wq": np.ascontiguousarray(
                wq_t[:, hs, :].reshape(D, 512).reshape(NK, P, 512)),
            "wk": np.ascontiguousarray(
                wk_t[:, hs, :].reshape(D, 512).reshape(NK, P, 512)),
            "wv": np.ascontiguousarray(
                wv_t[:, hs, :].reshape(D, 512).reshape(NK, P, 512)),
            "wpT": np.ascontiguousarray(
                Wp.T[512 * half:512 * half + 512, :].reshape(NPAIR, P, D)),
        })

    in_maps = []
    for c in range(8):
        b, half = c // 2, c % 2
        import ml_dtypes
        identr = ident.astype(ml_dtypes.bfloat16) if AV_BF16 else ident
        m = {"xT": np.ascontiguousarray(x[b].T), "mask": mask, "ident": ident,
             "identr": identr, "patt": patt}
        m.update(halves[half])
        in_maps.append(m)
    return in_maps


def kernel(x, Wq, Wk, Wv, Wp, bp, _results_hook=None):
    in_maps = make_in_maps(x, Wq, Wk, Wv, Wp)
    nc = build_program()
    kr = run_bass_kernel_spmd(nc, in_maps, core_ids=list(range(8)))
    if _results_hook is not None:
        _results_hook(kr)
    bp = np.asarray(bp, dtype=np.float32)
    out = np.empty((B, S, D), dtype=np.float32)
    for b in range(B):
        out[b] = kr.results[2 * b]["y"] + kr.results[2 * b + 1]["y"] + bp
    return out
